# revision 20
# baseline (speedup 1.0000x reference)
"""Trainium2 Bass kernel for BackgroundSubtractorModule.

Reference computation (per 15-frame window, gray video):
  y      = 0.299 R + 0.587 G + 0.114 B            (per pixel, x scale)
  m      = mean_f y ; var = sum_f (y-m)^2 / 14
  sigma  = sqrt(var) + 1e-5
  bg     = |y - m| / sigma
  minv/maxv = min/max over pixels of bg (per frame)
  out    = (bg - minv) / (maxv - minv)  if rng > 1e-6 else bg

Sharding: 30 independent windows across 8 cores; every core runs an
identical 4-window program (cores 6,7 process one duplicated pad window
whose output is dropped).

Design (trace-driven; v2):
  * minv of bg over 147456 pixels is ~1e-5 while rng ~4; dropping it
    contributes ~2e-6 rel error: out = |bg| / maxv.
  * fp16 pipeline after the rgb load; output quantized to uint8 (x254)
    halving store traffic; host divides by 254.
  * Luma: ACT makes the scaled R copy (3-frame batched, strided); one
    fused DVE scalar_tensor_tensor computes t2=(B*a_b)+G for 3 frames;
    one batched DVE add combines. Per-instruction fixed cost (~0.3us)
    amortizes 3x; fp16 tensor_tensor runs in DVE 2x mode.
  * PE accumulates sum(y) and sum(y^2) as fp16 identity-matmul PSUM
    accumulation (6 banks); the square runs on the otherwise-idle
    GPSIMD (Q7) one group behind the luma so nothing waits on it.
  * Variance via (sum(y^2)-15m^2)/14; 1/sigma' computed as DVE
    reciprocal_approx_fast (1 op, fp32, ~4e-6 rel) + ACT Sqrt into
    fp16 - replaces the 8.8us/window DVE InstReciprocal.
  * Pass 2: 3-frame batched subtract and multiply (mean / 1/sigma
    broadcast via stride-0 AP, both 2x fp16), then the 3-frame
    batched abs-max tensor_reduce (1x; abs_max as a TT AluOp does
    not survive walrus codegen, so no pairwise pre-fold).
  * Per-window max pipelined per 3-frame group (GPSIMD partition
    all-reduce + DVE approx-reciprocal + x254 per group) so
    normalize+stores start immediately - no window-wide barrier.
  * 3-frame batched 5.3 MB loads on sync HWDGE (20 loads/core); uint8
    stores on scalar HWDGE. rgb double-buffered.
"""

import numpy as np
from contextlib import ExitStack

import concourse.bass as bass
import concourse.bacc as bacc
import concourse.tile as tile
from concourse import mybir, bass_isa
from concourse.bass_utils import run_bass_kernel_spmd

F32 = mybir.dt.float32
F16 = mybir.dt.float16
OP = mybir.AluOpType
AF = mybir.ActivationFunctionType

T, H, W = 450, 384, 384
PIX = H * W                    # 147456
WIN = 15
NCORES = 8
NWIN_CORE = 4                  # ceil(30/8) -> uniform SPMD program
FPC = NWIN_CORE * WIN          # 60 frames per core
P = 128
COLS = PIX // P                # 1152
HCOLS = COLS // 2              # 576
EPS = 1e-5
BANKS = ((0, 512), (512, 1024), (1024, 1152))   # PSUM bank-aligned slices

# engine/batching knobs (A/B-able)
SQ_GPSIMD_GROUPS = 5           # groups/window with square on Q7 (rest ACT)
SUB_ON_GPSIMD = False           # d = y - m on Q7 (frees ~42us DVE)

_BUILD_CACHE = {}


def _build(scale: float):
    w0, w1, w2 = 0.299 * scale, 0.587 * scale, 0.114 * scale
    a_r, a_b = w0 / w1, w2 / w1
    nc = bacc.Bacc("TRN2", target_bir_lowering=False, debug=False)
    vin = nc.dram_tensor("video", [FPC, PIX * 3], F32, kind="ExternalInput").ap()
    idd = nc.dram_tensor("ident", [P, P], F16, kind="ExternalInput").ap()
    vout = nc.dram_tensor("out", [FPC, PIX], mybir.dt.uint8, kind="ExternalOutput").ap()

    with tile.TileContext(nc) as tc, ExitStack() as ctx:
        p_const = ctx.enter_context(tc.tile_pool(name="const", bufs=1))
        p_y = ctx.enter_context(tc.tile_pool(name="y", bufs=2))
        p_rgb = ctx.enter_context(tc.tile_pool(name="rgb", bufs=2))
        p_stat = ctx.enter_context(tc.tile_pool(name="stat", bufs=2))
        p_tmp = ctx.enter_context(tc.tile_pool(name="tmp", bufs=1))
        p_ftmp = ctx.enter_context(tc.tile_pool(name="ftmp", bufs=2))
        p_mm = ctx.enter_context(tc.tile_pool(name="mm", bufs=2))
        p_ps = ctx.enter_context(tc.tile_pool(name="psum", bufs=1, space="PSUM"))

        ident = p_const.tile([P, P], F16)
        nc.sync.dma_start(ident[:], idd[:])
        c254 = p_const.tile([P, 16], F32)
        nc.gpsimd.memset(c254[:], 254.0)

        st8 = {}
        pending_sq = []

        def flush_sq():
            while pending_sq:
                w, grp = pending_sq.pop(0)
                S = st8[w]
                f0 = grp * 3
                y3 = S["yt"][:, f0 * COLS:(f0 + 3) * COLS]
                sq = p_ftmp.tile([P, 3 * COLS], F16, tag="sq")
                if grp < SQ_GPSIMD_GROUPS:
                    nc.gpsimd.tensor_tensor(sq[:], y3, y3, OP.mult)
                else:
                    nc.scalar.activation(sq[:], y3, AF.Square)
                for k in range(3):
                    f = f0 + k
                    for lo, hi in BANKS:
                        nc.tensor.matmul(S["acc_q"][:, lo:hi], ident[:],
                                         sq[:, k * COLS + lo:k * COLS + hi],
                                         start=(f == 0), stop=(f == WIN - 1))

        def mk_state(w):
            st8[w] = dict(
                yt=p_y.tile([P, WIN * COLS], F16, tag="y", name=f"yt{w}"),
                acc_s=p_ps.tile([P, COLS], F32, tag="acc_s", name=f"accs{w}"),
                acc_q=p_ps.tile([P, COLS], F32, tag="acc_q", name=f"accq{w}"),
                mt=p_stat.tile([P, COLS], F16, tag="m", name=f"mt{w}"),
                ish=p_stat.tile([P, COLS], F16, tag="ish", name=f"ish{w}"),
                mmt=p_mm.tile([P, 48], F32, tag="mm", name=f"mmt{w}"),
            )

        def yslice(w, f):
            yt = st8[w]["yt"]
            return yt[:, f * COLS:(f + 1) * COLS]

        def load_group(w, grp):
            g = w * WIN + grp * 3
            rgbt = p_rgb.tile([P, 3 * COLS * 3], F32, tag="rgb")
            nc.sync.dma_start(
                rgbt[:].rearrange("p (f x) -> p f x", f=3),
                vin[g:g + 3].rearrange("f (r x) -> r f x", r=P))
            return rgbt

        def p1_group(w, grp, rgbt):
            S = st8[w]
            f0 = grp * 3
            rgb4 = rgbt[:].rearrange("p (f j c) -> p f j c", f=3, c=3)
            y3 = S["yt"][:, f0 * COLS:(f0 + 3) * COLS]
            y3v = y3.rearrange("p (f j) -> p f j", f=3)
            t2 = p_tmp.tile([P, 3 * COLS], F16, tag="t2")
            # R' for 3 frames in one strided ACT copy
            nc.scalar.activation(y3v, rgb4[:, :, :, 0], AF.Copy,
                                 bias=0.0, scale=a_r)
            # t2 = (B * a_b) + G for 3 frames in one fused DVE op
            nc.vector.scalar_tensor_tensor(
                t2[:].rearrange("p (f j) -> p f j", f=3),
                rgb4[:, :, :, 2], a_b, rgb4[:, :, :, 1], OP.mult, OP.add)
            nc.vector.tensor_tensor(y3, y3, t2[:], OP.add)       # fp16 2x
            for k in range(3):
                f = f0 + k
                yf = yslice(w, f)
                for lo, hi in BANKS:
                    nc.tensor.matmul(S["acc_s"][:, lo:hi], ident[:], yf[:, lo:hi],
                                     start=(f == 0), stop=(f == WIN - 1))
            # square+acc_q skewed one group: nothing waits on this
            # group's luma (head-of-line fix)
            flush_sq()
            pending_sq.append((w, grp))

        def p2(w):
            flush_sq()                       # emit the last group's square
            S = st8[w]
            # mean (fp16, for the subtract)
            nc.scalar.activation(S["mt"][:], S["acc_s"][:], AF.Copy,
                                 bias=0.0, scale=float(1.0 / WIN))
            # 15*m^2 = (sum/sqrt(15))^2, exact from psum
            msq = p_tmp.tile([P, COLS], F32, tag="msq")
            nc.scalar.activation(msq[:], S["acc_s"][:], AF.Square,
                                 scale=float(1.0 / np.sqrt(15.0)))
            # varsum = acc_q - 15 m^2 (in place into msq)
            nc.vector.tensor_tensor(msq[:], S["acc_q"][:], msq[:], OP.subtract)
            # 1/sigma' = sqrt(14/varsum): approx-reciprocal (1 DVE op,
            # ~4e-6 rel) then ACT sqrt into fp16. EPS contributes <4e-4.
            rv = p_tmp.tile([P, COLS], F32, tag="rv")
            nc.vector.reciprocal_approx_fast(rv[:], msq[:])
            nc.scalar.activation(S["ish"][:], rv[:], AF.Sqrt,
                                 scale=float(WIN - 1))

        def p3_group(w, grp):
            S = st8[w]
            f0 = grp * 3
            y3 = S["yt"][:, f0 * COLS:(f0 + 3) * COLS]
            y3v = y3.rearrange("p (f j) -> p f j", f=3)
            # d = y - m, bg = d * ish: 3-frame batched, stride-0 bcast
            mtb = S["mt"][:].unsqueeze(1).broadcast_to((P, 3, COLS))
            if SUB_ON_GPSIMD:
                nc.gpsimd.tensor_tensor(y3v, y3v, mtb, OP.subtract)
            else:
                nc.vector.tensor_tensor(y3v, y3v, mtb, OP.subtract)
            ishb = S["ish"][:].unsqueeze(1).broadcast_to((P, 3, COLS))
            nc.vector.tensor_tensor(y3v, y3v, ishb, OP.mult)
            # max|bg| per frame (abs folded into the 1x reduce)
            nc.vector.tensor_reduce(
                S["mmt"][:, f0:f0 + 3], y3v, axis=mybir.AxisListType.X,
                op=OP.max, apply_absolute_value=True)
            mmt = S["mmt"]
            nc.gpsimd.partition_all_reduce(
                mmt[:, 16 + f0:19 + f0], mmt[:, f0:f0 + 3], 128,
                bass_isa.ReduceOp.max)

        def nrm_group(w, grp):
            # 254/max: emitted a full group after its all-reduce so the
            # DVE recip never stalls on the gpsimd chain
            mmt = st8[w]["mmt"]
            f0 = grp * 3
            nc.vector.reciprocal_approx_fast(mmt[:, 32 + f0:35 + f0],
                                             mmt[:, 16 + f0:19 + f0])
            nc.vector.tensor_scalar(mmt[:, 32 + f0:35 + f0],
                                    mmt[:, 32 + f0:35 + f0], 254.0, None,
                                    OP.mult)

        def p5_group(w, grp):
            S = st8[w]
            mmt = S["mmt"]
            f0 = grp * 3
            ob = p_ftmp.tile([P, 3 * COLS], mybir.dt.uint8, tag="ob")
            for k in range(3):
                f = f0 + k
                # round(|bg| * 254/maxv) -> uint8
                nc.scalar.activation(
                    ob[:, k * COLS:(k + 1) * COLS], yslice(w, f), AF.Abs,
                    bias=0.0, scale=mmt[:, 32 + f:33 + f])
            g0 = w * WIN + f0
            nc.scalar.dma_start(
                vout[g0:g0 + 3].rearrange("f (r j) -> r f j", r=P),
                ob[:].rearrange("p (f j) -> p f j", f=3),
            )

        # ---- software-pipelined emission ----
        # Split skew: nrm(g) lands at the END of group g+1's DVE block
        # (a full group after its gpsimd all-reduce -> no DVE stall);
        # p5(g) flushes at the START of group g+2 (its scale is long
        # ready -> the ACT abs never blocks the R-copy behind it).
        pend_nrm = []   # groups normalized but not yet stored
        pend_p3 = []    # groups reduced but not yet normalized

        def flush_p5():
            while pend_nrm:
                p5_group(*pend_nrm.pop(0))

        def flush_nrm1():
            if pend_p3:
                pw, pg = pend_p3.pop(0)
                nrm_group(pw, pg)
                pend_nrm.append((pw, pg))

        mk_state(0)
        for grp in range(5):
            rgbt = load_group(0, grp)
            p1_group(0, grp, rgbt)
        for w in range(NWIN_CORE):
            nxt = w + 1 if w + 1 < NWIN_CORE else None
            if nxt is not None:
                mk_state(nxt)
            p2(w)
            for grp in range(5):
                flush_p5()
                p3_group(w, grp)
                pend_p3.append((w, grp))
                if nxt is not None:
                    rgbt = load_group(nxt, grp)
                    p1_group(nxt, grp, rgbt)
                flush_nrm1()
            # drain this window's pends before the next window's tiles
            # can recycle its yt slot
            while pend_p3:
                flush_nrm1()
            flush_p5()
        for w in list(st8):
            del st8[w]

    nc.compile()
    return nc


def _get_nc(scale: float):
    key = round(float(scale), 9)
    if key not in _BUILD_CACHE:
        _BUILD_CACHE[key] = _build(key)
    return _BUILD_CACHE[key]


def kernel(video: np.ndarray) -> np.ndarray:
    video = np.ascontiguousarray(np.asarray(video, dtype=np.float32))
    assert video.shape == (T, H, W, 3), video.shape
    scale = 1.0 / 255.0 if float(video.max()) > 1.0 else 1.0
    nc = _get_nc(scale)

    v = video.reshape(T, PIX * 3)
    shards = []
    for c in range(6):
        shards.append(v[c * FPC:(c + 1) * FPC])
    # cores 6,7: 3 real windows + last window repeated as pad
    shards.append(np.concatenate([v[360:405], v[390:405]], axis=0))
    shards.append(np.concatenate([v[405:450], v[435:450]], axis=0))

    ident = np.eye(P, dtype=np.float16)
    res = run_bass_kernel_spmd(
        nc, [{"video": s, "ident": ident} for s in shards], list(range(NCORES))
    )
    outs = [res.results[c]["out"].astype(np.float32) / np.float32(254.0)
            for c in range(NCORES)]
    full = np.concatenate(
        [o[:FPC] for o in outs[:6]] + [outs[6][:45], outs[7][:45]], axis=0
    )
    return full.reshape(T, 1, H, W)


# revision 27
# speedup vs baseline: 1.3205x; 1.3205x over previous
"""Trainium2 Bass kernel for BackgroundSubtractorModule.

Reference computation (per 15-frame window, gray video):
  y      = 0.299 R + 0.587 G + 0.114 B            (per pixel, x scale)
  m      = mean_f y ; var = sum_f (y-m)^2 / 14
  sigma  = sqrt(var) + 1e-5
  bg     = |y - m| / sigma
  minv/maxv = min/max over pixels of bg (per frame)
  out    = (bg - minv) / (maxv - minv)  if rng > 1e-6 else bg

Sharding: 30 independent windows across 8 cores; every core runs an
identical 4-window program (cores 6,7 process one duplicated pad window
whose output is dropped).

Design (trace-driven; v2):
  * minv of bg over 147456 pixels is ~1e-5 while rng ~4; dropping it
    contributes ~2e-6 rel error: out = |bg| / maxv.
  * fp16 pipeline after the rgb load; output quantized to uint8 (x254)
    halving store traffic; host divides by 254.
  * Luma: ACT makes the scaled R copy (3-frame batched, strided); one
    fused DVE scalar_tensor_tensor computes t2=(B*a_b)+G for 3 frames;
    one batched DVE add combines. Per-instruction fixed cost (~0.3us)
    amortizes 3x; fp16 tensor_tensor runs in DVE 2x mode.
  * PE accumulates sum(y) and sum(y^2) as fp16 identity-matmul PSUM
    accumulation (6 banks); the square runs on the otherwise-idle
    GPSIMD (Q7) one group behind the luma so nothing waits on it.
  * Variance via (sum(y^2)-15m^2)/14; 1/sigma' computed as DVE
    reciprocal_approx_fast (1 op, fp32, ~4e-6 rel) + ACT Sqrt into
    fp16 - replaces the 8.8us/window DVE InstReciprocal.
  * Pass 2: 3-frame batched subtract and multiply (mean / 1/sigma
    broadcast via stride-0 AP, both 2x fp16), then the 3-frame
    batched abs-max tensor_reduce (1x; abs_max as a TT AluOp does
    not survive walrus codegen, so no pairwise pre-fold).
  * Per-window max pipelined per 3-frame group (GPSIMD partition
    all-reduce + DVE approx-reciprocal + x254 per group) so
    normalize+stores start immediately - no window-wide barrier.
  * 3-frame batched 5.3 MB loads on sync HWDGE (20 loads/core); uint8
    stores on scalar HWDGE. rgb double-buffered.
"""

import numpy as np
from contextlib import ExitStack

import concourse.bass as bass
import concourse.bacc as bacc
import concourse.tile as tile
from concourse import mybir, bass_isa
from concourse.bass_utils import run_bass_kernel_spmd

F32 = mybir.dt.float32
F16 = mybir.dt.float16
OP = mybir.AluOpType
AF = mybir.ActivationFunctionType

T, H, W = 450, 384, 384
PIX = H * W                    # 147456
WIN = 15
NCORES = 8
NWIN_CORE = 4                  # ceil(30/8) -> uniform SPMD program
FPC = NWIN_CORE * WIN          # 60 frames per core
P = 128
COLS = PIX // P                # 1152
HCOLS = COLS // 2              # 576
EPS = 1e-5
BANKS = ((0, 512), (512, 1024), (1024, 1152))   # PSUM bank-aligned slices

# engine/batching knobs (A/B-able)
SQ_GPSIMD_GROUPS = 0           # groups/window with square on Q7 (rest ACT)
SUB_ON_GPSIMD = False          # d = y - m on Q7 (HW-corrupts w/ bcast: keep off)

_BUILD_CACHE = {}


def _build(scale: float):
    w0, w1, w2 = 0.299 * scale, 0.587 * scale, 0.114 * scale
    a_r, a_b = w0 / w1, w2 / w1
    nc = bacc.Bacc("TRN2", target_bir_lowering=False, debug=False)
    vin = nc.dram_tensor("video", [FPC, PIX * 3], F32, kind="ExternalInput").ap()
    idd = nc.dram_tensor("ident", [P, P], F16, kind="ExternalInput").ap()
    vout = nc.dram_tensor("out", [FPC, PIX], mybir.dt.uint8, kind="ExternalOutput").ap()

    with tile.TileContext(nc) as tc, ExitStack() as ctx:
        p_const = ctx.enter_context(tc.tile_pool(name="const", bufs=1))
        p_y = ctx.enter_context(tc.tile_pool(name="y", bufs=2))
        p_rgb = ctx.enter_context(tc.tile_pool(name="rgb", bufs=2))
        p_stat = ctx.enter_context(tc.tile_pool(name="stat", bufs=2))
        p_tmp = ctx.enter_context(tc.tile_pool(name="tmp", bufs=1))
        p_ftmp = ctx.enter_context(tc.tile_pool(name="ftmp", bufs=2))
        p_mm = ctx.enter_context(tc.tile_pool(name="mm", bufs=2))
        p_ps = ctx.enter_context(tc.tile_pool(name="psum", bufs=1, space="PSUM"))

        ident = p_const.tile([P, P], F16)
        nc.sync.dma_start(ident[:], idd[:])
        cln254 = p_const.tile([P, 1], F32)
        nc.vector.memset(cln254[:], float(np.log(254.0)))

        st8 = {}
        pending_sq = []

        def flush_sq():
            while pending_sq:
                w, grp = pending_sq.pop(0)
                S = st8[w]
                f0 = grp * 3
                y3 = S["yt"][:, f0 * COLS:(f0 + 3) * COLS]
                sq = p_ftmp.tile([P, 3 * COLS], F16, tag="sq")
                if grp < SQ_GPSIMD_GROUPS:
                    nc.gpsimd.tensor_tensor(sq[:], y3, y3, OP.mult)
                else:
                    nc.scalar.activation(sq[:], y3, AF.Square)
                for k in range(3):
                    f = f0 + k
                    for lo, hi in BANKS:
                        nc.tensor.matmul(S["acc_q"][:, lo:hi], ident[:],
                                         sq[:, k * COLS + lo:k * COLS + hi],
                                         start=(f == 0), stop=(f == WIN - 1))

        def mk_state(w):
            st8[w] = dict(
                yt=p_y.tile([P, WIN * COLS], F16, tag="y", name=f"yt{w}"),
                acc_s=p_ps.tile([P, COLS], F32, tag="acc_s", name=f"accs{w}"),
                acc_q=p_ps.tile([P, COLS], F32, tag="acc_q", name=f"accq{w}"),
                mt=p_stat.tile([P, COLS], F16, tag="m", name=f"mt{w}"),
                ish=p_stat.tile([P, COLS], F16, tag="ish", name=f"ish{w}"),
                mmt=p_mm.tile([P, 64], F32, tag="mm", name=f"mmt{w}"),
            )

        def yslice(w, f):
            yt = st8[w]["yt"]
            return yt[:, f * COLS:(f + 1) * COLS]

        def load_group(w, grp):
            g = w * WIN + grp * 3
            rgbt = p_rgb.tile([P, 3 * COLS * 3], F32, tag="rgb")
            nc.sync.dma_start(
                rgbt[:].rearrange("p (f x) -> p f x", f=3),
                vin[g:g + 3].rearrange("f (r x) -> r f x", r=P))
            return rgbt

        def p1_group(w, grp, rgbt):
            S = st8[w]
            f0 = grp * 3
            rgb4 = rgbt[:].rearrange("p (f j c) -> p f j c", f=3, c=3)
            y3 = S["yt"][:, f0 * COLS:(f0 + 3) * COLS]
            y3v = y3.rearrange("p (f j) -> p f j", f=3)
            t2 = p_tmp.tile([P, 3 * COLS], F16, tag="t2")
            # luma entirely on DVE (two fused stt ops) so the rgb
            # buffer release never sits behind the ACT abs queue:
            # t2 = B*a_b + G ; y = R*a_r + t2
            nc.vector.scalar_tensor_tensor(
                t2[:].rearrange("p (f j) -> p f j", f=3),
                rgb4[:, :, :, 2], a_b, rgb4[:, :, :, 1], OP.mult, OP.add)
            nc.vector.scalar_tensor_tensor(
                y3v, rgb4[:, :, :, 0], a_r,
                t2[:].rearrange("p (f j) -> p f j", f=3), OP.mult, OP.add)
            for k in range(3):
                f = f0 + k
                yf = yslice(w, f)
                for lo, hi in BANKS:
                    nc.tensor.matmul(S["acc_s"][:, lo:hi], ident[:], yf[:, lo:hi],
                                     start=(f == 0), stop=(f == WIN - 1))
            # square+acc_q skewed one group: nothing waits on this
            # group's luma (head-of-line fix)
            flush_sq()
            pending_sq.append((w, grp))

        def p2(w):
            flush_sq()                       # emit the last group's square
            S = st8[w]
            # mean (fp16, for the subtract)
            nc.scalar.activation(S["mt"][:], S["acc_s"][:], AF.Copy,
                                 bias=0.0, scale=float(1.0 / WIN))
            # 15*m^2 = (sum/sqrt(15))^2, exact from psum
            msq = p_tmp.tile([P, COLS], F32, tag="msq")
            nc.scalar.activation(msq[:], S["acc_s"][:], AF.Square,
                                 scale=float(1.0 / np.sqrt(15.0)))
            # varsum = acc_q - 15 m^2 (in place into msq)
            nc.vector.tensor_tensor(msq[:], S["acc_q"][:], msq[:], OP.subtract)
            # 1/sigma' = sqrt(14/varsum): approx-reciprocal (1 DVE op,
            # ~4e-6 rel) then ACT sqrt into fp16. EPS contributes <4e-4.
            rv = p_tmp.tile([P, COLS], F32, tag="rv")
            nc.vector.reciprocal_approx_fast(rv[:], msq[:])
            nc.scalar.activation(S["ish"][:], rv[:], AF.Sqrt,
                                 scale=float(WIN - 1))

        def p3_group(w, grp):
            S = st8[w]
            f0 = grp * 3
            y3 = S["yt"][:, f0 * COLS:(f0 + 3) * COLS]
            y3v = y3.rearrange("p (f j) -> p f j", f=3)
            # d = y - m, bg = d * ish: 3-frame batched, stride-0 bcast
            mtb = S["mt"][:].unsqueeze(1).broadcast_to((P, 3, COLS))
            if SUB_ON_GPSIMD:
                nc.gpsimd.tensor_tensor(y3v, y3v, mtb, OP.subtract)
            else:
                nc.vector.tensor_tensor(y3v, y3v, mtb, OP.subtract)
            ishb = S["ish"][:].unsqueeze(1).broadcast_to((P, 3, COLS))
            nc.vector.tensor_tensor(y3v, y3v, ishb, OP.mult)
            # max|bg| per frame (abs folded into the 1x reduce)
            nc.vector.tensor_reduce(
                S["mmt"][:, f0:f0 + 3], y3v, axis=mybir.AxisListType.X,
                op=OP.max, apply_absolute_value=True)
            mmt = S["mmt"]
            nc.gpsimd.partition_all_reduce(
                mmt[:, 16 + f0:19 + f0], mmt[:, f0:f0 + 3], 128,
                bass_isa.ReduceOp.max)

        def nrm_group(w, grp):
            # 254/max on ACT via exp(ln254 - ln M): keeps the whole
            # max->scale chain off the DVE (no cross-engine stall into
            # the load-release path). Table error ~1e-3 on the scale.
            mmt = st8[w]["mmt"]
            f0 = grp * 3
            nc.scalar.activation(mmt[:, 48 + f0:51 + f0],
                                 mmt[:, 16 + f0:19 + f0], AF.Ln)
            nc.scalar.activation(mmt[:, 32 + f0:35 + f0],
                                 mmt[:, 48 + f0:51 + f0], AF.Exp,
                                 bias=cln254[:], scale=-1.0)

        def p5_group(w, grp):
            S = st8[w]
            mmt = S["mmt"]
            f0 = grp * 3
            ob = p_ftmp.tile([P, 3 * COLS], mybir.dt.uint8, tag="ob")
            for k in range(3):
                f = f0 + k
                # round(|bg| * 254/maxv) -> uint8
                nc.scalar.activation(
                    ob[:, k * COLS:(k + 1) * COLS], yslice(w, f), AF.Abs,
                    bias=0.0, scale=mmt[:, 32 + f:33 + f])
            g0 = w * WIN + f0
            nc.scalar.dma_start(
                vout[g0:g0 + 3].rearrange("f (r j) -> r f j", r=P),
                ob[:].rearrange("p (f j) -> p f j", f=3),
            )

        # ---- software-pipelined emission ----
        mk_state(0)
        for grp in range(5):
            rgbt = load_group(0, grp)
            p1_group(0, grp, rgbt)
        for w in range(NWIN_CORE):
            nxt = w + 1 if w + 1 < NWIN_CORE else None
            if nxt is not None:
                mk_state(nxt)
            p2(w)
            for grp in range(5):
                p3_group(w, grp)
                nrm_group(w, grp)
                p5_group(w, grp)
                if nxt is not None:
                    rgbt = load_group(nxt, grp)
                    p1_group(nxt, grp, rgbt)
            del st8[w]

    nc.compile()
    return nc


def _get_nc(scale: float):
    key = round(float(scale), 9)
    if key not in _BUILD_CACHE:
        _BUILD_CACHE[key] = _build(key)
    return _BUILD_CACHE[key]


def kernel(video: np.ndarray) -> np.ndarray:
    video = np.ascontiguousarray(np.asarray(video, dtype=np.float32))
    assert video.shape == (T, H, W, 3), video.shape
    scale = 1.0 / 255.0 if float(video.max()) > 1.0 else 1.0
    nc = _get_nc(scale)

    v = video.reshape(T, PIX * 3)
    shards = []
    for c in range(6):
        shards.append(v[c * FPC:(c + 1) * FPC])
    # cores 6,7: 3 real windows + last window repeated as pad
    shards.append(np.concatenate([v[360:405], v[390:405]], axis=0))
    shards.append(np.concatenate([v[405:450], v[435:450]], axis=0))

    ident = np.eye(P, dtype=np.float16)
    res = run_bass_kernel_spmd(
        nc, [{"video": s, "ident": ident} for s in shards], list(range(NCORES))
    )
    outs = [res.results[c]["out"].astype(np.float32) / np.float32(254.0)
            for c in range(NCORES)]
    full = np.concatenate(
        [o[:FPC] for o in outs[:6]] + [outs[6][:45], outs[7][:45]], axis=0
    )
    return full.reshape(T, 1, H, W)
